# revision 4
# baseline (speedup 1.0000x reference)
"""Bahdanau-style additive attention on 8 TRN2 NeuronCores.

score(n, l) = v . tanh(decoder_hidden[n] @ W_h.T + encoder_hiddens[n, l] @ W_s.T)
attn = softmax(mask(score));  context[n] = attn[n] @ encoder_hiddens[n]

Sharding: data-parallel over batch N=64 -> 8 batches per core, weights
replicated, no collectives.

Device layouts (prepared host-side so the device never transposes the big
encoder tensor):
  eT   [8, H, L] bf16  - per-batch transposed encoder (for the W_s matmul,
                         contraction over h must sit on SBUF partitions)
  eN   [8, L, H] bf16  - natural layout (for the context matmul, contraction
                         over l on partitions)
  wsT  [H, H]    bf16  - W_s.T            whT [H, H] bf16 - W_h.T
  decT [H, 8]    bf16  - decoder shard transposed
  vcol [128, 8]  bf16  - v reshaped so chunk c lives at [:, c]
  mneg [8, L]    f32   - -1e30 where PAD else 0
Compute: bf16 matmuls with f32 PSUM accumulation, f32 softmax.
"""

import os
import numpy as np
import ml_dtypes

N_CORES = 8
N, L, H = 64, 1024, 1024
NB = N // N_CORES  # batches per core
P = 128
HC = H // P  # h chunks
LC = L // P  # l chunks
KC = H // P  # k (output-dim) chunks

_cache = {}

last_exec_time_ns = None
last_trace = None


def _build():
    import concourse.bass as bass
    import concourse.bacc as bacc
    import concourse.tile as tile
    from concourse import mybir

    f32 = mybir.dt.float32
    bf16 = mybir.dt.bfloat16
    TANH = mybir.ActivationFunctionType.Tanh
    EXP = mybir.ActivationFunctionType.Exp
    COPY = mybir.ActivationFunctionType.Copy
    AXX = mybir.AxisListType.X

    nc = bacc.Bacc("TRN2", target_bir_lowering=False, debug=False,
                   num_devices=N_CORES)

    eT = nc.dram_tensor("eT", [NB, H, L], bf16, kind="ExternalInput")
    eN = nc.dram_tensor("eN", [NB, L, H], bf16, kind="ExternalInput")
    wsT = nc.dram_tensor("wsT", [H, H], bf16, kind="ExternalInput")
    whT = nc.dram_tensor("whT", [H, H], bf16, kind="ExternalInput")
    decT = nc.dram_tensor("decT", [H, NB], bf16, kind="ExternalInput")
    vcol = nc.dram_tensor("vcol", [P, HC], bf16, kind="ExternalInput")
    mneg = nc.dram_tensor("mneg", [NB, L], f32, kind="ExternalInput")
    ident = nc.dram_tensor("ident", [NB, NB], bf16, kind="ExternalInput")
    ctx_out = nc.dram_tensor("ctx", [NB, H], f32, kind="ExternalOutput")
    attn_out = nc.dram_tensor("attn", [NB, L], f32, kind="ExternalOutput")

    with tile.TileContext(nc) as tc:
        with (
            tc.tile_pool(name="const", bufs=1) as cpool,
            tc.tile_pool(name="et", bufs=2) as etpool,
            tc.tile_pool(name="en", bufs=2) as enpool,
            tc.tile_pool(name="work", bufs=3) as wpool,
            tc.tile_pool(name="ps", bufs=2, space=bass.MemorySpace.PSUM) as ppool,
            tc.tile_pool(name="ps1", bufs=1, space=bass.MemorySpace.PSUM) as ppool1,
        ):
            # ---- load replicated weights ----
            ws_sb = cpool.tile([P, HC, H], bf16)   # [p, hc, k] = wsT[hc*P+p, k]
            wh_sb = cpool.tile([P, HC, H], bf16)
            dec_sb = cpool.tile([P, HC, NB], bf16)
            for hc in range(HC):
                nc.sync.dma_start(ws_sb[:, hc, :], wsT[hc * P:(hc + 1) * P, :])
                nc.sync.dma_start(wh_sb[:, hc, :], whT[hc * P:(hc + 1) * P, :])
                nc.sync.dma_start(dec_sb[:, hc, :], decT[hc * P:(hc + 1) * P, :])
            v_sb = cpool.tile([P, HC], bf16)
            nc.sync.dma_start(v_sb[:], vcol[:, :])
            mneg_sb = cpool.tile([NB, L], f32)
            nc.sync.dma_start(mneg_sb[:], mneg[:, :])

            # ---- dh^T[k, n] = sum_h W_h[k, h] * dec[n, h] ----
            dhT_sb = cpool.tile([P, KC, NB], f32)
            for kc in range(KC):
                ps = ppool.tile([P, NB], f32, tag="small")
                for hc in range(HC):
                    nc.tensor.matmul(
                        ps[:],
                        wh_sb[:, hc, kc * P:(kc + 1) * P],
                        dec_sb[:, hc, :],
                        start=(hc == 0), stop=(hc == HC - 1))
                nc.vector.tensor_copy(dhT_sb[:, kc, :], ps[:])

            # ---- phase 1: scores ----
            score_all = cpool.tile([NB, L], f32)
            for n in range(NB):
                et_sb = etpool.tile([P, HC, L], bf16, tag="et")
                for hc in range(HC):
                    nc.sync.dma_start(et_sb[:, hc, :],
                                      eT[n, hc * P:(hc + 1) * P, :])
                sc_ps = ppool1.tile([1, L], f32, tag="row")
                for kc in range(KC):
                    eh_ps = ppool.tile([P, L], f32, tag="ehps")
                    for hc in range(HC):
                        for lt in range(2):
                            nc.tensor.matmul(
                                eh_ps[:, lt * 512:(lt + 1) * 512],
                                ws_sb[:, hc, kc * P:(kc + 1) * P],
                                et_sb[:, hc, lt * 512:(lt + 1) * 512],
                                start=(hc == 0), stop=(hc == HC - 1))
                    th = wpool.tile([P, L], bf16, tag="tanh")
                    nc.scalar.activation(th[:], eh_ps[:], TANH,
                                         bias=dhT_sb[:, kc, n:n + 1])
                    for lt in range(2):
                        nc.tensor.matmul(
                            sc_ps[:, lt * 512:(lt + 1) * 512],
                            v_sb[:, kc:kc + 1],
                            th[:, lt * 512:(lt + 1) * 512],
                            start=(kc == 0), stop=(kc == KC - 1))
                # stash the score row (DMA may target any partition,
                # engine APs may not -> bounce through partition 0)
                sc_row = wpool.tile([1, L], f32, tag="scrow")
                nc.vector.tensor_copy(sc_row[:], sc_ps[:])
                nc.sync.dma_start(score_all[n:n + 1, :], sc_row[:])

            # ---- phase 2: batched softmax over l (free dim) ----
            score_m = cpool.tile([NB, L], f32)
            nc.vector.tensor_add(score_m[:], score_all[:], mneg_sb[:])
            mx = cpool.tile([NB, 1], f32)
            nc.vector.reduce_max(mx[:], score_m[:], axis=AXX)
            negmax = cpool.tile([NB, 1], f32)
            nc.vector.tensor_scalar_mul(negmax[:], mx[:], -1.0)
            prob = cpool.tile([NB, L], f32)
            zsum = cpool.tile([NB, 1], f32)
            nc.scalar.activation(prob[:], score_m[:], EXP,
                                 bias=negmax[:], accum_out=zsum[:])
            rz = cpool.tile([NB, 1], f32)
            nc.vector.reciprocal(rz[:], zsum[:])
            attn_f = cpool.tile([NB, L], f32)
            nc.scalar.activation(attn_f[:], prob[:], COPY, scale=rz[:])
            attn_b = cpool.tile([NB, L], bf16)
            nc.vector.tensor_scalar_mul(attn_b[:], prob[:], rz[:])
            nc.sync.dma_start(attn_out[:, :], attn_f[:])

            # ---- transpose attn rows -> columns for all batches at once:
            # acolT[p, lc, n] = attn[n, lc*P+p] via matmul with I8 ----
            i8 = cpool.tile([NB, NB], bf16)
            nc.sync.dma_start(i8[:], ident[:, :])
            acolT = cpool.tile([P, LC, NB], bf16)
            for lc in range(LC):
                ac_ps = ppool.tile([P, NB], f32, tag="small")
                nc.tensor.matmul(ac_ps[:], attn_b[:, lc * P:(lc + 1) * P],
                                 i8[:], start=True, stop=True)
                nc.vector.tensor_copy(acolT[:, lc, :], ac_ps[:])

            # ---- phase 3: context[n, h] = sum_l attn[n, l] E[n, l, h] ----
            for n in range(NB):
                en_sb = enpool.tile([P, LC, H], bf16, tag="en")
                for lc in range(LC):
                    nc.sync.dma_start(en_sb[:, lc, :],
                                      eN[n, lc * P:(lc + 1) * P, :])
                cx_ps = ppool1.tile([1, H], f32, tag="row")
                for lc in range(LC):
                    for ht in range(2):
                        nc.tensor.matmul(
                            cx_ps[:, ht * 512:(ht + 1) * 512],
                            acolT[:, lc, n:n + 1],
                            en_sb[:, lc, ht * 512:(ht + 1) * 512],
                            start=(lc == 0), stop=(lc == LC - 1))
                cx_row = wpool.tile([1, H], f32, tag="cxrow")
                nc.vector.tensor_copy(cx_row[:], cx_ps[:])
                nc.sync.dma_start(ctx_out[n:n + 1, :], cx_row[:])

    nc.compile()
    return nc


def kernel(decoder_hidden, encoder_hiddens, mask, W_h, W_s, v):
    global last_exec_time_ns, last_trace
    from concourse.bass_utils import run_bass_kernel_spmd

    bf16 = ml_dtypes.bfloat16
    dec = np.asarray(decoder_hidden, np.float32)
    enc = np.asarray(encoder_hiddens, np.float32)
    msk = np.asarray(mask)
    W_h = np.asarray(W_h, np.float32)
    W_s = np.asarray(W_s, np.float32)
    v = np.asarray(v, np.float32)

    wsT = np.ascontiguousarray(W_s.T).astype(bf16)
    whT = np.ascontiguousarray(W_h.T).astype(bf16)
    vcol = np.ascontiguousarray(v.reshape(HC, P).T).astype(bf16)
    mneg = np.where(msk, np.float32(-1e30), np.float32(0.0)).astype(np.float32)

    enc_b = enc.astype(bf16)

    in_maps = []
    for c in range(N_CORES):
        s = slice(c * NB, (c + 1) * NB)
        in_maps.append({
            "eT": np.ascontiguousarray(enc_b[s].transpose(0, 2, 1)),
            "eN": np.ascontiguousarray(enc_b[s]),
            "wsT": wsT,
            "whT": whT,
            "decT": np.ascontiguousarray(dec[s].T).astype(bf16),
            "vcol": vcol,
            "mneg": np.ascontiguousarray(mneg[s]),
            "ident": np.eye(NB, dtype=bf16),
        })

    if "nc" not in _cache:
        _cache["nc"] = _build()
    nc = _cache["nc"]

    trace = bool(int(os.environ.get("BASS_KERNEL_TRACE", "0")))
    res = run_bass_kernel_spmd(nc, in_maps, core_ids=list(range(N_CORES)),
                               trace=trace)
    last_exec_time_ns = res.exec_time_ns
    last_trace = res.instructions_and_trace

    context = np.concatenate([res.results[c]["ctx"] for c in range(N_CORES)], 0)
    attn_w = np.concatenate([res.results[c]["attn"] for c in range(N_CORES)], 0)
    return (context.astype(np.float32), attn_w.astype(np.float32))


# revision 5
# speedup vs baseline: 1.0785x; 1.0785x over previous
"""Bahdanau-style additive attention on 8 TRN2 NeuronCores.

score(n, l) = v . tanh(decoder_hidden[n] @ W_h.T + encoder_hiddens[n, l] @ W_s.T)
attn = softmax(mask(score));  context[n] = attn[n] @ encoder_hiddens[n]

Sharding: data-parallel over batch N=64 -> 8 batches per core, weights
replicated, no collectives.

Device layouts (prepared host-side so the device never transposes the big
encoder tensor):
  eT   [8, H, L] bf16  - per-batch transposed encoder (for the W_s matmul,
                         contraction over h must sit on SBUF partitions)
  eN   [8, L, H] bf16  - natural layout (for the context matmul, contraction
                         over l on partitions)
  wsT  [H, H]    bf16  - W_s.T            whT [H, H] bf16 - W_h.T
  decT [H, 8]    bf16  - decoder shard transposed
  vcol [128, 8]  bf16  - v reshaped so chunk c lives at [:, c]
  mneg [8, L]    f32   - -1e30 where PAD else 0
Compute: bf16 matmuls with f32 PSUM accumulation, f32 softmax.
"""

import os
import numpy as np
import ml_dtypes

N_CORES = 8
N, L, H = 64, 1024, 1024
NB = N // N_CORES  # batches per core
P = 128
HC = H // P  # h chunks
LC = L // P  # l chunks
KC = H // P  # k (output-dim) chunks

_cache = {}

last_exec_time_ns = None
last_trace = None


def _build():
    import concourse.bass as bass
    import concourse.bacc as bacc
    import concourse.tile as tile
    from concourse import mybir

    f32 = mybir.dt.float32
    bf16 = mybir.dt.bfloat16
    TANH = mybir.ActivationFunctionType.Tanh
    EXP = mybir.ActivationFunctionType.Exp
    COPY = mybir.ActivationFunctionType.Copy
    AXX = mybir.AxisListType.X

    nc = bacc.Bacc("TRN2", target_bir_lowering=False, debug=False,
                   num_devices=N_CORES)

    eT = nc.dram_tensor("eT", [NB, H, L], bf16, kind="ExternalInput")
    eN = nc.dram_tensor("eN", [NB, L, H], bf16, kind="ExternalInput")
    wsT = nc.dram_tensor("wsT", [H, H], bf16, kind="ExternalInput")
    whT = nc.dram_tensor("whT", [H, H], bf16, kind="ExternalInput")
    decT = nc.dram_tensor("decT", [H, NB], bf16, kind="ExternalInput")
    vcol = nc.dram_tensor("vcol", [P, HC], bf16, kind="ExternalInput")
    mneg = nc.dram_tensor("mneg", [1, NB, L], f32, kind="ExternalInput")
    ctx_out = nc.dram_tensor("ctx", [NB, H], f32, kind="ExternalOutput")
    attn_out = nc.dram_tensor("attn", [NB, L], f32, kind="ExternalOutput")

    with tile.TileContext(nc) as tc:
        with (
            tc.tile_pool(name="const", bufs=1) as cpool,
            tc.tile_pool(name="et", bufs=2) as etpool,
            tc.tile_pool(name="en", bufs=2) as enpool,
            tc.tile_pool(name="work", bufs=3) as wpool,
            tc.tile_pool(name="ps", bufs=2, space=bass.MemorySpace.PSUM) as ppool,
            tc.tile_pool(name="ps1", bufs=1, space=bass.MemorySpace.PSUM) as ppool1,
        ):
            # ---- load replicated weights ----
            ws_sb = cpool.tile([P, HC, H], bf16)   # [p, hc, k] = wsT[hc*P+p, k]
            wh_sb = cpool.tile([P, HC, H], bf16)
            dec_sb = cpool.tile([P, HC, NB], bf16)
            for hc in range(HC):
                nc.sync.dma_start(ws_sb[:, hc, :], wsT[hc * P:(hc + 1) * P, :])
                nc.sync.dma_start(wh_sb[:, hc, :], whT[hc * P:(hc + 1) * P, :])
                nc.sync.dma_start(dec_sb[:, hc, :], decT[hc * P:(hc + 1) * P, :])
            v_sb = cpool.tile([P, HC], bf16)
            nc.sync.dma_start(v_sb[:], vcol[:, :])
            mneg_sb = cpool.tile([1, NB, L], f32)
            nc.sync.dma_start(mneg_sb[:], mneg[:, :, :])

            # ---- dh^T[k, n] = sum_h W_h[k, h] * dec[n, h] ----
            dhT_sb = cpool.tile([P, KC, NB], f32)
            for kc in range(KC):
                ps = ppool.tile([P, NB], f32, tag="ehps")
                for hc in range(HC):
                    nc.tensor.matmul(
                        ps[:],
                        wh_sb[:, hc, kc * P:(kc + 1) * P],
                        dec_sb[:, hc, :],
                        start=(hc == 0), stop=(hc == HC - 1))
                nc.vector.tensor_copy(dhT_sb[:, kc, :], ps[:])

            # ---- fused per-batch pipeline ----
            ones_sb = cpool.tile([1, 1], bf16)
            nc.vector.memset(ones_sb[:], 1.0)
            for n in range(NB):
                et_sb = etpool.tile([P, HC, L], bf16, tag="et")
                for hc in range(HC):
                    nc.sync.dma_start(et_sb[:, hc, :],
                                      eT[n, hc * P:(hc + 1) * P, :])
                en_sb = enpool.tile([P, LC, H], bf16, tag="en")
                for lc in range(LC):
                    nc.sync.dma_start(en_sb[:, lc, :],
                                      eN[n, lc * P:(lc + 1) * P, :])

                # scores: ehT[k, l] = sum_h Ws[k, h] E[l, h]; v . tanh(+dh)
                sc_ps = ppool1.tile([1, L], f32, tag="row")
                for kc in range(KC):
                    eh_ps = ppool.tile([P, L], f32, tag="ehps")
                    for hc in range(HC):
                        for lt in range(2):
                            nc.tensor.matmul(
                                eh_ps[:, lt * 512:(lt + 1) * 512],
                                ws_sb[:, hc, kc * P:(kc + 1) * P],
                                et_sb[:, hc, lt * 512:(lt + 1) * 512],
                                start=(hc == 0), stop=(hc == HC - 1))
                    th = wpool.tile([P, L], bf16, tag="tanh")
                    nc.scalar.activation(th[:], eh_ps[:], TANH,
                                         bias=dhT_sb[:, kc, n:n + 1])
                    for lt in range(2):
                        nc.tensor.matmul(
                            sc_ps[:, lt * 512:(lt + 1) * 512],
                            v_sb[:, kc:kc + 1],
                            th[:, lt * 512:(lt + 1) * 512],
                            start=(kc == 0), stop=(kc == KC - 1))

                # per-batch masked softmax on a single partition-0 row
                sc_row = wpool.tile([1, L], f32, tag="scrow")
                nc.vector.tensor_add(sc_row[:], sc_ps[:], mneg_sb[0:1, n, :])
                mx = wpool.tile([1, 1], f32, tag="mx")
                nc.vector.reduce_max(mx[:], sc_row[:], axis=AXX)
                ngm = wpool.tile([1, 1], f32, tag="ngm")
                nc.vector.tensor_scalar_mul(ngm[:], mx[:], -1.0)
                prob = wpool.tile([1, L], f32, tag="prob")
                z = wpool.tile([1, 1], f32, tag="z")
                nc.scalar.activation(prob[:], sc_row[:], EXP,
                                     bias=ngm[:], accum_out=z[:])
                rz = wpool.tile([1, 1], f32, tag="rz")
                nc.vector.reciprocal(rz[:], z[:])
                arow_f = wpool.tile([1, L], f32, tag="arowf")
                nc.vector.tensor_scalar_mul(arow_f[:], prob[:], rz[:])
                nc.sync.dma_start(attn_out[n:n + 1, :], arow_f[:])
                arow_b = wpool.tile([1, L], bf16, tag="arowb")
                nc.vector.tensor_scalar_mul(arow_b[:], prob[:], rz[:])

                # transpose attn row -> columns via outer products with 1
                ac_ps = ppool1.tile([P, LC], f32, tag="pc")
                for lc in range(LC):
                    nc.tensor.matmul(ac_ps[:, lc:lc + 1],
                                     arow_b[0:1, lc * P:(lc + 1) * P],
                                     ones_sb[:], start=True, stop=True)
                acol = wpool.tile([P, LC], bf16, tag="acol")
                nc.vector.tensor_copy(acol[:], ac_ps[:])

                # context[n, h] = sum_l attn[l] E[l, h]
                cx_ps = ppool1.tile([1, H], f32, tag="pc")
                for lc in range(LC):
                    for ht in range(2):
                        nc.tensor.matmul(
                            cx_ps[:, ht * 512:(ht + 1) * 512],
                            acol[:, lc:lc + 1],
                            en_sb[:, lc, ht * 512:(ht + 1) * 512],
                            start=(lc == 0), stop=(lc == LC - 1))
                cx_row = wpool.tile([1, H], f32, tag="cxrow")
                nc.vector.tensor_copy(cx_row[:], cx_ps[:])
                nc.sync.dma_start(ctx_out[n:n + 1, :], cx_row[:])

    nc.compile()
    return nc


def kernel(decoder_hidden, encoder_hiddens, mask, W_h, W_s, v):
    global last_exec_time_ns, last_trace
    from concourse.bass_utils import run_bass_kernel_spmd

    bf16 = ml_dtypes.bfloat16
    dec = np.asarray(decoder_hidden, np.float32)
    enc = np.asarray(encoder_hiddens, np.float32)
    msk = np.asarray(mask)
    W_h = np.asarray(W_h, np.float32)
    W_s = np.asarray(W_s, np.float32)
    v = np.asarray(v, np.float32)

    wsT = np.ascontiguousarray(W_s.T).astype(bf16)
    whT = np.ascontiguousarray(W_h.T).astype(bf16)
    vcol = np.ascontiguousarray(v.reshape(HC, P).T).astype(bf16)
    mneg = np.where(msk, np.float32(-1e30), np.float32(0.0)).astype(np.float32)

    enc_b = enc.astype(bf16)

    in_maps = []
    for c in range(N_CORES):
        s = slice(c * NB, (c + 1) * NB)
        in_maps.append({
            "eT": np.ascontiguousarray(enc_b[s].transpose(0, 2, 1)),
            "eN": np.ascontiguousarray(enc_b[s]),
            "wsT": wsT,
            "whT": whT,
            "decT": np.ascontiguousarray(dec[s].T).astype(bf16),
            "vcol": vcol,
            "mneg": np.ascontiguousarray(mneg[s]).reshape(1, NB, L),
        })

    if "nc" not in _cache:
        _cache["nc"] = _build()
    nc = _cache["nc"]

    trace = bool(int(os.environ.get("BASS_KERNEL_TRACE", "0")))
    res = run_bass_kernel_spmd(nc, in_maps, core_ids=list(range(N_CORES)),
                               trace=trace)
    last_exec_time_ns = res.exec_time_ns
    last_trace = res.instructions_and_trace

    context = np.concatenate([res.results[c]["ctx"] for c in range(N_CORES)], 0)
    attn_w = np.concatenate([res.results[c]["attn"] for c in range(N_CORES)], 0)
    return (context.astype(np.float32), attn_w.astype(np.float32))


# revision 6
# speedup vs baseline: 1.1542x; 1.0702x over previous
"""Bahdanau-style additive attention on 8 TRN2 NeuronCores.

score(n, l) = v . tanh(decoder_hidden[n] @ W_h.T + encoder_hiddens[n, l] @ W_s.T)
attn = softmax(mask(score));  context[n] = attn[n] @ encoder_hiddens[n]

Sharding: data-parallel over batch N=64 -> 8 batches per core, weights
replicated, no collectives.

Device layouts (prepared host-side so the device never transposes the big
encoder tensor):
  eT   [8, H, L] bf16  - per-batch transposed encoder (for the W_s matmul,
                         contraction over h must sit on SBUF partitions)
  eN   [8, L, H] bf16  - natural layout (for the context matmul, contraction
                         over l on partitions)
  wsT  [H, H]    bf16  - W_s.T            whT [H, H] bf16 - W_h.T
  decT [H, 8]    bf16  - decoder shard transposed
  vcol [128, 8]  bf16  - v reshaped so chunk c lives at [:, c]
  mneg [8, L]    f32   - -1e30 where PAD else 0
Compute: bf16 matmuls with f32 PSUM accumulation, f32 softmax.
"""

import os
import numpy as np
import ml_dtypes

N_CORES = 8
N, L, H = 64, 1024, 1024
NB = N // N_CORES  # batches per core
P = 128
HC = H // P  # h chunks
LC = L // P  # l chunks
KC = H // P  # k (output-dim) chunks

_cache = {}

last_exec_time_ns = None
last_trace = None


def _build():
    import concourse.bass as bass
    import concourse.bacc as bacc
    import concourse.tile as tile
    from concourse import mybir

    f32 = mybir.dt.float32
    bf16 = mybir.dt.bfloat16
    TANH = mybir.ActivationFunctionType.Tanh
    EXP = mybir.ActivationFunctionType.Exp
    COPY = mybir.ActivationFunctionType.Copy
    AXX = mybir.AxisListType.X

    nc = bacc.Bacc("TRN2", target_bir_lowering=False, debug=False,
                   num_devices=N_CORES)

    eT = nc.dram_tensor("eT", [NB, H, L], bf16, kind="ExternalInput")
    eN = nc.dram_tensor("eN", [NB, L, H], bf16, kind="ExternalInput")
    wsT = nc.dram_tensor("wsT", [H, H], bf16, kind="ExternalInput")
    whT = nc.dram_tensor("whT", [H, H], bf16, kind="ExternalInput")
    decT = nc.dram_tensor("decT", [H, NB], bf16, kind="ExternalInput")
    vcol = nc.dram_tensor("vcol", [P, HC], bf16, kind="ExternalInput")
    mneg = nc.dram_tensor("mneg", [1, NB, L], f32, kind="ExternalInput")
    ctx_out = nc.dram_tensor("ctx", [NB, H], f32, kind="ExternalOutput")
    attn_out = nc.dram_tensor("attn", [NB, L], f32, kind="ExternalOutput")

    with tile.TileContext(nc) as tc:
        with (
            tc.tile_pool(name="const", bufs=1) as cpool,
            tc.tile_pool(name="et", bufs=2) as etpool,
            tc.tile_pool(name="en", bufs=2) as enpool,
            tc.tile_pool(name="work", bufs=3) as wpool,
            tc.tile_pool(name="ps", bufs=2, space=bass.MemorySpace.PSUM) as ppool,
            tc.tile_pool(name="ps1", bufs=1, space=bass.MemorySpace.PSUM) as ppool1,
        ):
            # ---- load replicated weights; order shapes DMA queue order:
            # dec+wh first (feeds dh matmuls), then ws / et(n=0)
            # interleaved so the main matmul stream can start early ----
            ws_sb = cpool.tile([P, HC, H], bf16)   # [p, hc, k] = wsT[hc*P+p, k]
            wh_sb = cpool.tile([P, HC, H], bf16)
            dec_sb = cpool.tile([P, HC, NB], bf16)
            for hc in range(HC):
                nc.sync.dma_start(dec_sb[:, hc, :], decT[hc * P:(hc + 1) * P, :])
            for hc in range(HC):
                nc.sync.dma_start(wh_sb[:, hc, :], whT[hc * P:(hc + 1) * P, :])
            v_sb = cpool.tile([P, HC], bf16)
            nc.sync.dma_start(v_sb[:], vcol[:, :])
            mneg_sb = cpool.tile([1, NB, L], f32)
            nc.sync.dma_start(mneg_sb[:], mneg[:, :, :])
            et0_sb = etpool.tile([P, HC, L], bf16, tag="et")
            for hc in range(HC):
                nc.sync.dma_start(ws_sb[:, hc, :], wsT[hc * P:(hc + 1) * P, :])
                nc.sync.dma_start(et0_sb[:, hc, :], eT[0, hc * P:(hc + 1) * P, :])

            # ---- dh^T[k, n] = sum_h W_h[k, h] * dec[n, h] ----
            dhT_sb = cpool.tile([P, KC, NB], f32)
            for kc in range(KC):
                ps = ppool.tile([P, NB], f32, tag="ehps")
                for hc in range(HC):
                    nc.tensor.matmul(
                        ps[:],
                        wh_sb[:, hc, kc * P:(kc + 1) * P],
                        dec_sb[:, hc, :],
                        start=(hc == 0), stop=(hc == HC - 1))
                nc.vector.tensor_copy(dhT_sb[:, kc, :], ps[:])

            # ---- fused per-batch pipeline ----
            ones_sb = cpool.tile([1, 1], bf16)
            nc.vector.memset(ones_sb[:], 1.0)
            for n in range(NB):
                if n == 0:
                    et_sb = et0_sb
                else:
                    et_sb = etpool.tile([P, HC, L], bf16, tag="et")
                    for hc in range(HC):
                        nc.sync.dma_start(et_sb[:, hc, :],
                                          eT[n, hc * P:(hc + 1) * P, :])
                en_sb = enpool.tile([P, LC, H], bf16, tag="en")
                for lc in range(LC):
                    nc.sync.dma_start(en_sb[:, lc, :],
                                      eN[n, lc * P:(lc + 1) * P, :])

                # scores: ehT[k, l] = sum_h Ws[k, h] E[l, h]; v . tanh(+dh)
                sc_ps = ppool1.tile([1, L], f32, tag="row")
                for kc in range(KC):
                    eh_ps = ppool.tile([P, L], f32, tag="ehps")
                    for hc in range(HC):
                        for lt in range(2):
                            nc.tensor.matmul(
                                eh_ps[:, lt * 512:(lt + 1) * 512],
                                ws_sb[:, hc, kc * P:(kc + 1) * P],
                                et_sb[:, hc, lt * 512:(lt + 1) * 512],
                                start=(hc == 0), stop=(hc == HC - 1))
                    th = wpool.tile([P, L], bf16, tag="tanh")
                    nc.scalar.activation(th[:], eh_ps[:], TANH,
                                         bias=dhT_sb[:, kc, n:n + 1])
                    for lt in range(2):
                        nc.tensor.matmul(
                            sc_ps[:, lt * 512:(lt + 1) * 512],
                            v_sb[:, kc:kc + 1],
                            th[:, lt * 512:(lt + 1) * 512],
                            start=(kc == 0), stop=(kc == KC - 1))

                # per-batch masked softmax on a single partition-0 row
                sc_row = wpool.tile([1, L], f32, tag="scrow")
                nc.vector.tensor_add(sc_row[:], sc_ps[:], mneg_sb[0:1, n, :])
                mx = wpool.tile([1, 1], f32, tag="mx")
                nc.vector.reduce_max(mx[:], sc_row[:], axis=AXX)
                ngm = wpool.tile([1, 1], f32, tag="ngm")
                nc.vector.tensor_scalar_mul(ngm[:], mx[:], -1.0)
                prob = wpool.tile([1, L], f32, tag="prob")
                z = wpool.tile([1, 1], f32, tag="z")
                nc.scalar.activation(prob[:], sc_row[:], EXP,
                                     bias=ngm[:], accum_out=z[:])
                rz = wpool.tile([1, 1], f32, tag="rz")
                nc.vector.reciprocal(rz[:], z[:])
                arow_b = wpool.tile([1, L], bf16, tag="arowb")
                nc.vector.tensor_scalar_mul(arow_b[:], prob[:], rz[:])
                arow_f = wpool.tile([1, L], f32, tag="arowf")
                nc.vector.tensor_scalar_mul(arow_f[:], prob[:], rz[:])
                nc.sync.dma_start(attn_out[n:n + 1, :], arow_f[:])

                # transpose attn row -> columns via outer products with 1
                ac_ps = ppool1.tile([P, LC], f32, tag="pc")
                for lc in range(LC):
                    nc.tensor.matmul(ac_ps[:, lc:lc + 1],
                                     arow_b[0:1, lc * P:(lc + 1) * P],
                                     ones_sb[:], start=True, stop=True)
                acol = wpool.tile([P, LC], bf16, tag="acol")
                nc.vector.tensor_copy(acol[:], ac_ps[:])

                # context[n, h] = sum_l attn[l] E[l, h]; 4 column groups
                # compute disjoint h-quarters concurrently (tile_position)
                cx_ps = ppool1.tile([P, H], f32, tag="pc")
                Q = H // 4
                for lc in range(LC):
                    for j in range(4):
                        nc.tensor.matmul(
                            cx_ps[32 * j:32 * j + 1, j * Q:(j + 1) * Q],
                            acol[:, lc:lc + 1],
                            en_sb[:, lc, j * Q:(j + 1) * Q],
                            start=(lc == 0), stop=(lc == LC - 1),
                            tile_position=(0, 32 * j))
                cx_row = wpool.tile([P, H], f32, tag="cxrow")
                nc.vector.tensor_copy(cx_row[:], cx_ps[:])
                for j in range(4):
                    nc.sync.dma_start(ctx_out[n:n + 1, j * Q:(j + 1) * Q],
                                      cx_row[32 * j:32 * j + 1, j * Q:(j + 1) * Q])

    nc.compile()
    return nc


def kernel(decoder_hidden, encoder_hiddens, mask, W_h, W_s, v):
    global last_exec_time_ns, last_trace
    from concourse.bass_utils import run_bass_kernel_spmd

    bf16 = ml_dtypes.bfloat16
    dec = np.asarray(decoder_hidden, np.float32)
    enc = np.asarray(encoder_hiddens, np.float32)
    msk = np.asarray(mask)
    W_h = np.asarray(W_h, np.float32)
    W_s = np.asarray(W_s, np.float32)
    v = np.asarray(v, np.float32)

    wsT = np.ascontiguousarray(W_s.T).astype(bf16)
    whT = np.ascontiguousarray(W_h.T).astype(bf16)
    vcol = np.ascontiguousarray(v.reshape(HC, P).T).astype(bf16)
    mneg = np.where(msk, np.float32(-1e30), np.float32(0.0)).astype(np.float32)

    enc_b = enc.astype(bf16)

    in_maps = []
    for c in range(N_CORES):
        s = slice(c * NB, (c + 1) * NB)
        in_maps.append({
            "eT": np.ascontiguousarray(enc_b[s].transpose(0, 2, 1)),
            "eN": np.ascontiguousarray(enc_b[s]),
            "wsT": wsT,
            "whT": whT,
            "decT": np.ascontiguousarray(dec[s].T).astype(bf16),
            "vcol": vcol,
            "mneg": np.ascontiguousarray(mneg[s]).reshape(1, NB, L),
        })

    if "nc" not in _cache:
        _cache["nc"] = _build()
    nc = _cache["nc"]

    trace = bool(int(os.environ.get("BASS_KERNEL_TRACE", "0")))
    res = run_bass_kernel_spmd(nc, in_maps, core_ids=list(range(N_CORES)),
                               trace=trace)
    last_exec_time_ns = res.exec_time_ns
    last_trace = res.instructions_and_trace

    context = np.concatenate([res.results[c]["ctx"] for c in range(N_CORES)], 0)
    attn_w = np.concatenate([res.results[c]["attn"] for c in range(N_CORES)], 0)
    return (context.astype(np.float32), attn_w.astype(np.float32))


# revision 8
# speedup vs baseline: 1.2234x; 1.0600x over previous
"""Bahdanau-style additive attention on 8 TRN2 NeuronCores.

score(n, l) = v . tanh(decoder_hidden[n] @ W_h.T + encoder_hiddens[n, l] @ W_s.T)
attn = softmax(mask(score));  context[n] = attn[n] @ encoder_hiddens[n]

Sharding: data-parallel over batch N=64 -> 8 batches per core, weights
replicated, no collectives.

Device layouts (prepared host-side so the device never transposes the big
encoder tensor):
  eT   [8, H, L] bf16  - per-batch transposed encoder (for the W_s matmul,
                         contraction over h must sit on SBUF partitions)
  eN   [8, L, H] bf16  - natural layout (for the context matmul, contraction
                         over l on partitions)
  wsT  [H, H]    bf16  - W_s.T            whT [H, H] bf16 - W_h.T
  decT [H, 8]    bf16  - decoder shard transposed
  vcol [128, 8]  bf16  - v reshaped so chunk c lives at [:, c]
  mneg [8, L]    f32   - -1e30 where PAD else 0
Compute: bf16 matmuls with f32 PSUM accumulation, f32 softmax.
"""

import os
import numpy as np
import ml_dtypes

N_CORES = 8
N, L, H = 64, 1024, 1024
NB = N // N_CORES  # batches per core
P = 128
HC = H // P  # h chunks
LC = L // P  # l chunks
KC = H // P  # k (output-dim) chunks

_cache = {}

last_exec_time_ns = None
last_trace = None


def _build():
    import concourse.bass as bass
    import concourse.bacc as bacc
    import concourse.tile as tile
    from concourse import mybir

    f32 = mybir.dt.float32
    bf16 = mybir.dt.bfloat16
    TANH = mybir.ActivationFunctionType.Tanh
    EXP = mybir.ActivationFunctionType.Exp
    COPY = mybir.ActivationFunctionType.Copy
    AXX = mybir.AxisListType.X

    nc = bacc.Bacc("TRN2", target_bir_lowering=False, debug=False,
                   num_devices=N_CORES)

    eT = nc.dram_tensor("eT", [NB, H, L], bf16, kind="ExternalInput")
    eN = nc.dram_tensor("eN", [NB, L, H], bf16, kind="ExternalInput")
    wsT = nc.dram_tensor("wsT", [H, H], bf16, kind="ExternalInput")
    whT = nc.dram_tensor("whT", [H, H], bf16, kind="ExternalInput")
    decT = nc.dram_tensor("decT", [H, NB], bf16, kind="ExternalInput")
    vcol = nc.dram_tensor("vcol", [P, HC], bf16, kind="ExternalInput")
    mneg = nc.dram_tensor("mneg", [NB, P, L], f32, kind="ExternalInput")
    ctx_out = nc.dram_tensor("ctx", [NB, H], f32, kind="ExternalOutput")
    attn_out = nc.dram_tensor("attn", [NB, L], f32, kind="ExternalOutput")

    with tile.TileContext(nc) as tc:
        with (
            tc.tile_pool(name="const", bufs=1) as cpool,
            tc.tile_pool(name="et", bufs=2) as etpool,
            tc.tile_pool(name="en", bufs=2) as enpool,
            tc.tile_pool(name="work", bufs=3) as wpool,
            tc.tile_pool(name="rows", bufs=2) as rpool,
            tc.tile_pool(name="ps", bufs=2, space=bass.MemorySpace.PSUM) as ppool,
            tc.tile_pool(name="ps1", bufs=1, space=bass.MemorySpace.PSUM) as ppool1,
        ):
            # ---- load replicated weights; order shapes DMA queue order:
            # dec+wh first (feeds dh matmuls), then ws / et(n=0)
            # interleaved so the main matmul stream can start early ----
            ws_sb = cpool.tile([P, HC, H], bf16)   # [p, hc, k] = wsT[hc*P+p, k]
            wh_sb = cpool.tile([P, HC, H], bf16)
            dec_sb = cpool.tile([P, HC, NB], bf16)
            for hc in range(HC):
                nc.sync.dma_start(dec_sb[:, hc, :], decT[hc * P:(hc + 1) * P, :])
            for hc in range(HC):
                nc.sync.dma_start(wh_sb[:, hc, :], whT[hc * P:(hc + 1) * P, :])
            v_sb = cpool.tile([P, HC], bf16)
            nc.sync.dma_start(v_sb[:], vcol[:, :])

            et0_sb = etpool.tile([P, HC, L], bf16, tag="et")
            for hc in range(HC):
                nc.sync.dma_start(ws_sb[:, hc, :], wsT[hc * P:(hc + 1) * P, :])
                nc.sync.dma_start(et0_sb[:, hc, :], eT[0, hc * P:(hc + 1) * P, :])

            # ---- dh^T[k, n] = sum_h W_h[k, h] * dec[n, h] ----
            dhT_sb = cpool.tile([P, KC, NB], f32)
            for kc in range(KC):
                ps = ppool.tile([P, NB], f32, tag="ehps")
                for hc in range(HC):
                    nc.tensor.matmul(
                        ps[:],
                        wh_sb[:, hc, kc * P:(kc + 1) * P],
                        dec_sb[:, hc, :],
                        start=(hc == 0), stop=(hc == HC - 1))
                nc.vector.tensor_copy(dhT_sb[:, kc, :], ps[:])

            # ---- fused per-batch pipeline ----
            # ones on every partition (outer-product rhs for any row base)
            ones_sb = cpool.tile([P, 1], bf16)
            nc.vector.memset(ones_sb[:], 1.0)
            # Z-broadcast selector: ones at partitions {0,32,64,96} -> matmul
            # broadcasts the sum of the 4 per-quarter softmax sums to all
            # 128 output partitions
            selbc_sb = cpool.tile([P, P], f32)
            nc.vector.memset(selbc_sb[:], 0.0)
            for j in range(4):
                nc.vector.memset(selbc_sb[32 * j:32 * j + 1, :], 1.0)
            # per-batch mask tiles in split-row layout
            mneg_sb_all = cpool.tile([P, NB, L], f32)
            for n in range(NB):
                nc.sync.dma_start(mneg_sb_all[:, n, :], mneg[n, :, :])
            # scrub the score PSUM slot once: quarters only ever write their
            # 4 rows; stale bits elsewhere must not be NaN/huge (exp reads
            # the full tile)
            sc_init = ppool1.tile([P, L], f32, tag="row")
            nc.vector.memset(sc_init[:], 0.0)
            for n in range(NB):
                if n == 0:
                    et_sb = et0_sb
                else:
                    et_sb = etpool.tile([P, HC, L], bf16, tag="et")
                    for hc in range(HC):
                        nc.sync.dma_start(et_sb[:, hc, :],
                                          eT[n, hc * P:(hc + 1) * P, :])
                en_sb = enpool.tile([P, LC, H], bf16, tag="en")
                for lc in range(LC):
                    nc.sync.dma_start(en_sb[:, lc, :],
                                      eN[n, lc * P:(lc + 1) * P, :])

                # scores: ehT[k, l] = sum_h Ws[k, h] E[l, h]; v . tanh(+dh)
                sc_ps = ppool1.tile([P, L], f32, tag="row")
                QL = L // 4
                for kc in range(KC):
                    eh_ps = ppool.tile([P, L], f32, tag="ehps")
                    for hc in range(HC):
                        for lt in range(2):
                            nc.tensor.matmul(
                                eh_ps[:, lt * 512:(lt + 1) * 512],
                                ws_sb[:, hc, kc * P:(kc + 1) * P],
                                et_sb[:, hc, lt * 512:(lt + 1) * 512],
                                start=(hc == 0), stop=(hc == HC - 1))
                    th = wpool.tile([P, L], bf16, tag="tanh")
                    nc.scalar.activation(th[:], eh_ps[:], TANH,
                                         bias=dhT_sb[:, kc, n:n + 1])
                    # score quarters: column group j -> psum row 32j
                    for j in range(4):
                        nc.tensor.matmul(
                            sc_ps[32 * j:32 * j + 1, j * QL:(j + 1) * QL],
                            v_sb[:, kc:kc + 1],
                            th[:, j * QL:(j + 1) * QL],
                            start=(kc == 0), stop=(kc == KC - 1),
                            tile_position=(0, 32 * j))

                # masked softmax, no max-subtraction (|score| <= sum|v|
                # ~ 26, exp stays in f32 range; mask adds -1e30 pre-exp)
                sc_m = rpool.tile([P, L], f32, tag="scrow")
                nc.vector.tensor_add(sc_m[:], sc_ps[:], mneg_sb_all[:, n, :])
                prob = rpool.tile([P, L], f32, tag="prob")
                zs4 = wpool.tile([P, 1], f32, tag="z4")
                nc.scalar.activation(prob[:], sc_m[:], EXP, accum_out=zs4[:])
                z_ps = ppool1.tile([P, 1], f32, tag="pc")
                nc.tensor.matmul(z_ps[:], selbc_sb[:], zs4[:],
                                 start=True, stop=True)
                rzb = wpool.tile([P, 1], f32, tag="rz")
                nc.vector.reciprocal(rzb[:], z_ps[:])
                arow_b = wpool.tile([P, L], bf16, tag="arowb")
                nc.vector.tensor_scalar_mul(arow_b[:], prob[:], rzb[:])
                arow_f = rpool.tile([P, L], f32, tag="arowf")
                nc.vector.tensor_scalar_mul(arow_f[:], prob[:], rzb[:])
                for j in range(4):
                    nc.sync.dma_start(
                        attn_out[n:n + 1, j * QL:(j + 1) * QL],
                        arow_f[32 * j:32 * j + 1, j * QL:(j + 1) * QL])

                # transpose attn quarters -> columns via outer products
                ac_ps = ppool1.tile([P, LC], f32, tag="pc")
                for lc in range(LC):
                    j = lc // 2
                    nc.tensor.matmul(ac_ps[:, lc:lc + 1],
                                     arow_b[32 * j:32 * j + 1,
                                            lc * P:(lc + 1) * P],
                                     ones_sb[32 * j:32 * j + 1, :],
                                     start=True, stop=True,
                                     tile_position=(32 * j, 0))
                acol = wpool.tile([P, LC], bf16, tag="acol")
                nc.vector.tensor_copy(acol[:], ac_ps[:])

                # context[n, h] = sum_l attn[l] E[l, h]; 4 column groups
                # compute disjoint h-quarters concurrently (tile_position)
                cx_ps = ppool1.tile([P, H], f32, tag="pc")
                Q = H // 4
                for lc in range(LC):
                    for j in range(4):
                        nc.tensor.matmul(
                            cx_ps[32 * j:32 * j + 1, j * Q:(j + 1) * Q],
                            acol[:, lc:lc + 1],
                            en_sb[:, lc, j * Q:(j + 1) * Q],
                            start=(lc == 0), stop=(lc == LC - 1),
                            tile_position=(0, 32 * j))
                cx_row = rpool.tile([P, H], f32, tag="cxrow")
                nc.vector.tensor_copy(cx_row[:], cx_ps[:])
                for j in range(4):
                    nc.sync.dma_start(ctx_out[n:n + 1, j * Q:(j + 1) * Q],
                                      cx_row[32 * j:32 * j + 1, j * Q:(j + 1) * Q])

    nc.compile()
    return nc


def kernel(decoder_hidden, encoder_hiddens, mask, W_h, W_s, v):
    global last_exec_time_ns, last_trace
    from concourse.bass_utils import run_bass_kernel_spmd

    bf16 = ml_dtypes.bfloat16
    dec = np.asarray(decoder_hidden, np.float32)
    enc = np.asarray(encoder_hiddens, np.float32)
    msk = np.asarray(mask)
    W_h = np.asarray(W_h, np.float32)
    W_s = np.asarray(W_s, np.float32)
    v = np.asarray(v, np.float32)

    wsT = np.ascontiguousarray(W_s.T).astype(bf16)
    whT = np.ascontiguousarray(W_h.T).astype(bf16)
    vcol = np.ascontiguousarray(v.reshape(HC, P).T).astype(bf16)
    NEG = np.float32(-1e30)
    mneg_rows = np.where(msk, NEG, np.float32(0.0)).astype(np.float32)  # [N, L]
    QL = L // 4
    mneg4 = np.full((N, P, L), NEG, np.float32)
    for j in range(4):
        mneg4[:, 32 * j, j * QL:(j + 1) * QL] = \
            mneg_rows[:, j * QL:(j + 1) * QL]

    enc_b = enc.astype(bf16)

    in_maps = []
    for c in range(N_CORES):
        s = slice(c * NB, (c + 1) * NB)
        in_maps.append({
            "eT": np.ascontiguousarray(enc_b[s].transpose(0, 2, 1)),
            "eN": np.ascontiguousarray(enc_b[s]),
            "wsT": wsT,
            "whT": whT,
            "decT": np.ascontiguousarray(dec[s].T).astype(bf16),
            "vcol": vcol,
            "mneg": np.ascontiguousarray(mneg4[s]),
        })

    if "nc" not in _cache:
        _cache["nc"] = _build()
    nc = _cache["nc"]

    trace = bool(int(os.environ.get("BASS_KERNEL_TRACE", "0")))
    res = run_bass_kernel_spmd(nc, in_maps, core_ids=list(range(N_CORES)),
                               trace=trace)
    last_exec_time_ns = res.exec_time_ns
    last_trace = res.instructions_and_trace

    context = np.concatenate([res.results[c]["ctx"] for c in range(N_CORES)], 0)
    attn_w = np.concatenate([res.results[c]["attn"] for c in range(N_CORES)], 0)
    return (context.astype(np.float32), attn_w.astype(np.float32))


# revision 9
# speedup vs baseline: 1.2288x; 1.0044x over previous
"""Bahdanau-style additive attention on 8 TRN2 NeuronCores.

score(n, l) = v . tanh(decoder_hidden[n] @ W_h.T + encoder_hiddens[n, l] @ W_s.T)
attn = softmax(mask(score));  context[n] = attn[n] @ encoder_hiddens[n]

Sharding: data-parallel over batch N=64 -> 8 batches per core, weights
replicated, no collectives.

Device layouts (prepared host-side so the device never transposes the big
encoder tensor):
  eT   [8, H, L] bf16  - per-batch transposed encoder (for the W_s matmul,
                         contraction over h must sit on SBUF partitions)
  eN   [8, L, H] bf16  - natural layout (for the context matmul, contraction
                         over l on partitions)
  wsT  [H, H]    bf16  - W_s.T            whT [H, H] bf16 - W_h.T
  decT [H, 8]    bf16  - decoder shard transposed
  vcol [128, 8]  bf16  - v reshaped so chunk c lives at [:, c]
  mneg [8, L]    f32   - -1e30 where PAD else 0
Compute: bf16 matmuls with f32 PSUM accumulation, f32 softmax.
"""

import os
import numpy as np
import ml_dtypes

N_CORES = 8
N, L, H = 64, 1024, 1024
NB = N // N_CORES  # batches per core
P = 128
HC = H // P  # h chunks
LC = L // P  # l chunks
KC = H // P  # k (output-dim) chunks

_cache = {}

last_exec_time_ns = None
last_trace = None


def _build():
    import concourse.bass as bass
    import concourse.bacc as bacc
    import concourse.tile as tile
    from concourse import mybir

    f32 = mybir.dt.float32
    bf16 = mybir.dt.bfloat16
    TANH = mybir.ActivationFunctionType.Tanh
    EXP = mybir.ActivationFunctionType.Exp
    COPY = mybir.ActivationFunctionType.Copy
    AXX = mybir.AxisListType.X

    nc = bacc.Bacc("TRN2", target_bir_lowering=False, debug=False,
                   num_devices=N_CORES)

    eT = nc.dram_tensor("eT", [NB, H, L], bf16, kind="ExternalInput")
    eN = nc.dram_tensor("eN", [NB, L, H], bf16, kind="ExternalInput")
    wsT = nc.dram_tensor("wsT", [H, H], bf16, kind="ExternalInput")
    whT = nc.dram_tensor("whT", [H, H], bf16, kind="ExternalInput")
    decT = nc.dram_tensor("decT", [H, NB], bf16, kind="ExternalInput")
    vcol = nc.dram_tensor("vcol", [P, HC], bf16, kind="ExternalInput")
    mneg = nc.dram_tensor("mneg", [NB, P, L], f32, kind="ExternalInput")
    ctx_out = nc.dram_tensor("ctx", [NB, H], f32, kind="ExternalOutput")
    attn_out = nc.dram_tensor("attn", [NB, L], f32, kind="ExternalOutput")

    with tile.TileContext(nc) as tc:
        with (
            tc.tile_pool(name="const", bufs=1) as cpool,
            tc.tile_pool(name="et", bufs=2) as etpool,
            tc.tile_pool(name="en", bufs=2) as enpool,
            tc.tile_pool(name="work", bufs=3) as wpool,
            tc.tile_pool(name="rows", bufs=2) as rpool,
            tc.tile_pool(name="ps", bufs=2, space=bass.MemorySpace.PSUM) as ppool,
            tc.tile_pool(name="ps1", bufs=1, space=bass.MemorySpace.PSUM) as ppool1,
        ):
            # ---- load replicated weights; order shapes DMA queue order:
            # dec+wh first (feeds dh matmuls), then ws / et(n=0)
            # interleaved so the main matmul stream can start early ----
            ws_sb = cpool.tile([P, HC, H], bf16)   # [p, hc, k] = wsT[hc*P+p, k]
            wh_sb = cpool.tile([P, HC, H], bf16)
            dec_sb = cpool.tile([P, HC, NB], bf16)
            for hc in range(HC):
                nc.sync.dma_start(dec_sb[:, hc, :], decT[hc * P:(hc + 1) * P, :])
            for hc in range(HC):
                nc.sync.dma_start(wh_sb[:, hc, :], whT[hc * P:(hc + 1) * P, :])
            v_sb = cpool.tile([P, HC], bf16)
            nc.sync.dma_start(v_sb[:], vcol[:, :])

            et0_sb = etpool.tile([P, HC, L], bf16, tag="et")
            for hc in range(HC):
                nc.sync.dma_start(ws_sb[:, hc, :], wsT[hc * P:(hc + 1) * P, :])
                nc.sync.dma_start(et0_sb[:, hc, :], eT[0, hc * P:(hc + 1) * P, :])

            # ---- PE warmup: ~4us of dense dummy matmuls so the HAM
            # clock gate reaches 8/8 before the real stream starts ----
            warm_sb = cpool.tile([P, P], bf16)
            nc.vector.memset(warm_sb[:], 0.0)
            warm_ps = ppool1.tile([P, P], f32, tag="pc")
            for i in range(40):
                nc.tensor.matmul(warm_ps[:], warm_sb[:], warm_sb[:],
                                 start=True, stop=True)

            # ---- dh^T[k, n] = sum_h W_h[k, h] * dec[n, h] ----
            dhT_sb = cpool.tile([P, KC, NB], f32)
            for kc in range(KC):
                ps = ppool.tile([P, NB], f32, tag="ehps")
                for hc in range(HC):
                    nc.tensor.matmul(
                        ps[:],
                        wh_sb[:, hc, kc * P:(kc + 1) * P],
                        dec_sb[:, hc, :],
                        start=(hc == 0), stop=(hc == HC - 1))
                nc.vector.tensor_copy(dhT_sb[:, kc, :], ps[:])

            # ---- fused per-batch pipeline ----
            # ones on every partition (outer-product rhs for any row base)
            ones_sb = cpool.tile([P, 1], bf16)
            nc.vector.memset(ones_sb[:], 1.0)
            # Z-broadcast selector: ones at partitions {0,32,64,96} -> matmul
            # broadcasts the sum of the 4 per-quarter softmax sums to all
            # 128 output partitions
            selbc_sb = cpool.tile([P, P], f32)
            nc.vector.memset(selbc_sb[:], 0.0)
            for j in range(4):
                nc.vector.memset(selbc_sb[32 * j:32 * j + 1, :], 1.0)
            # per-batch mask tiles in split-row layout
            mneg_sb_all = cpool.tile([P, NB, L], f32)
            for n in range(NB):
                nc.sync.dma_start(mneg_sb_all[:, n, :], mneg[n, :, :])
            # scrub the score PSUM slot once: quarters only ever write their
            # 4 rows; stale bits elsewhere must not be NaN/huge (exp reads
            # the full tile)
            sc_init = ppool1.tile([P, L], f32, tag="row")
            nc.vector.memset(sc_init[:], 0.0)
            for n in range(NB):
                if n == 0:
                    et_sb = et0_sb
                else:
                    et_sb = etpool.tile([P, HC, L], bf16, tag="et")
                    for hc in range(HC):
                        nc.sync.dma_start(et_sb[:, hc, :],
                                          eT[n, hc * P:(hc + 1) * P, :])
                en_sb = enpool.tile([P, LC, H], bf16, tag="en")
                for lc in range(LC):
                    nc.sync.dma_start(en_sb[:, lc, :],
                                      eN[n, lc * P:(lc + 1) * P, :])

                # scores: ehT[k, l] = sum_h Ws[k, h] E[l, h]; v . tanh(+dh)
                sc_ps = ppool1.tile([P, L], f32, tag="row")
                QL = L // 4

                def sc_quads(kc, th):
                    # score quarters: column group j -> psum row 32j
                    for j in range(4):
                        nc.tensor.matmul(
                            sc_ps[32 * j:32 * j + 1, j * QL:(j + 1) * QL],
                            v_sb[:, kc:kc + 1],
                            th[:, j * QL:(j + 1) * QL],
                            start=(kc == 0), stop=(kc == KC - 1),
                            tile_position=(0, 32 * j))

                prev_th = None
                for kc in range(KC):
                    eh_ps = ppool.tile([P, L], f32, tag="ehps")
                    for hc in range(HC):
                        for lt in range(2):
                            nc.tensor.matmul(
                                eh_ps[:, lt * 512:(lt + 1) * 512],
                                ws_sb[:, hc, kc * P:(kc + 1) * P],
                                et_sb[:, hc, lt * 512:(lt + 1) * 512],
                                start=(hc == 0), stop=(hc == HC - 1))
                    th = wpool.tile([P, L], bf16, tag="tanh")
                    nc.scalar.activation(th[:], eh_ps[:], TANH,
                                         bias=dhT_sb[:, kc, n:n + 1])
                    if prev_th is not None:
                        sc_quads(kc - 1, prev_th)
                    prev_th = th
                sc_quads(KC - 1, prev_th)

                # masked softmax, no max-subtraction (|score| <= sum|v|
                # ~ 26, exp stays in f32 range; mask adds -1e30 pre-exp)
                sc_m = rpool.tile([P, L], f32, tag="scrow")
                nc.vector.tensor_add(sc_m[:], sc_ps[:], mneg_sb_all[:, n, :])
                prob = rpool.tile([P, L], f32, tag="prob")
                zs4 = wpool.tile([P, 1], f32, tag="z4")
                nc.scalar.activation(prob[:], sc_m[:], EXP, accum_out=zs4[:])
                z_ps = ppool1.tile([P, 1], f32, tag="pc")
                nc.tensor.matmul(z_ps[:], selbc_sb[:], zs4[:],
                                 start=True, stop=True)
                rzb = wpool.tile([P, 1], f32, tag="rz")
                nc.vector.reciprocal(rzb[:], z_ps[:])
                arow_b = wpool.tile([P, L], bf16, tag="arowb")
                nc.vector.tensor_scalar_mul(arow_b[:], prob[:], rzb[:])
                arow_f = rpool.tile([P, L], f32, tag="arowf")
                nc.vector.tensor_scalar_mul(arow_f[:], prob[:], rzb[:])
                for j in range(4):
                    nc.sync.dma_start(
                        attn_out[n:n + 1, j * QL:(j + 1) * QL],
                        arow_f[32 * j:32 * j + 1, j * QL:(j + 1) * QL])

                # transpose attn quarters -> columns via outer products
                ac_ps = ppool1.tile([P, LC], f32, tag="pc")
                for lc in range(LC):
                    j = lc // 2
                    nc.tensor.matmul(ac_ps[:, lc:lc + 1],
                                     arow_b[32 * j:32 * j + 1,
                                            lc * P:(lc + 1) * P],
                                     ones_sb[32 * j:32 * j + 1, :],
                                     start=True, stop=True,
                                     tile_position=(32 * j, 0))
                acol = wpool.tile([P, LC], bf16, tag="acol")
                nc.vector.tensor_copy(acol[:], ac_ps[:])

                # context[n, h] = sum_l attn[l] E[l, h]; 4 column groups
                # compute disjoint h-quarters concurrently (tile_position)
                cx_ps = ppool1.tile([P, H], f32, tag="pc")
                Q = H // 4
                for lc in range(LC):
                    for j in range(4):
                        nc.tensor.matmul(
                            cx_ps[32 * j:32 * j + 1, j * Q:(j + 1) * Q],
                            acol[:, lc:lc + 1],
                            en_sb[:, lc, j * Q:(j + 1) * Q],
                            start=(lc == 0), stop=(lc == LC - 1),
                            tile_position=(0, 32 * j))
                cx_row = rpool.tile([P, H], f32, tag="cxrow")
                nc.vector.tensor_copy(cx_row[:], cx_ps[:])
                for j in range(4):
                    nc.sync.dma_start(ctx_out[n:n + 1, j * Q:(j + 1) * Q],
                                      cx_row[32 * j:32 * j + 1, j * Q:(j + 1) * Q])

    nc.compile()
    return nc


def kernel(decoder_hidden, encoder_hiddens, mask, W_h, W_s, v):
    global last_exec_time_ns, last_trace
    from concourse.bass_utils import run_bass_kernel_spmd

    bf16 = ml_dtypes.bfloat16
    dec = np.asarray(decoder_hidden, np.float32)
    enc = np.asarray(encoder_hiddens, np.float32)
    msk = np.asarray(mask)
    W_h = np.asarray(W_h, np.float32)
    W_s = np.asarray(W_s, np.float32)
    v = np.asarray(v, np.float32)

    wsT = np.ascontiguousarray(W_s.T).astype(bf16)
    whT = np.ascontiguousarray(W_h.T).astype(bf16)
    vcol = np.ascontiguousarray(v.reshape(HC, P).T).astype(bf16)
    NEG = np.float32(-1e30)
    mneg_rows = np.where(msk, NEG, np.float32(0.0)).astype(np.float32)  # [N, L]
    QL = L // 4
    mneg4 = np.full((N, P, L), NEG, np.float32)
    for j in range(4):
        mneg4[:, 32 * j, j * QL:(j + 1) * QL] = \
            mneg_rows[:, j * QL:(j + 1) * QL]

    enc_b = enc.astype(bf16)

    in_maps = []
    for c in range(N_CORES):
        s = slice(c * NB, (c + 1) * NB)
        in_maps.append({
            "eT": np.ascontiguousarray(enc_b[s].transpose(0, 2, 1)),
            "eN": np.ascontiguousarray(enc_b[s]),
            "wsT": wsT,
            "whT": whT,
            "decT": np.ascontiguousarray(dec[s].T).astype(bf16),
            "vcol": vcol,
            "mneg": np.ascontiguousarray(mneg4[s]),
        })

    if "nc" not in _cache:
        _cache["nc"] = _build()
    nc = _cache["nc"]

    trace = bool(int(os.environ.get("BASS_KERNEL_TRACE", "0")))
    res = run_bass_kernel_spmd(nc, in_maps, core_ids=list(range(N_CORES)),
                               trace=trace)
    last_exec_time_ns = res.exec_time_ns
    last_trace = res.instructions_and_trace

    context = np.concatenate([res.results[c]["ctx"] for c in range(N_CORES)], 0)
    attn_w = np.concatenate([res.results[c]["attn"] for c in range(N_CORES)], 0)
    return (context.astype(np.float32), attn_w.astype(np.float32))
